# revision 31
# baseline (speedup 1.0000x reference)
"""Trainium2 Bass kernel for BertSelfAttention (B=1, S=4096, HID=768, 12 heads).

Sharding: 8 cores = 4 head-groups x 2 query-halves. Each core computes 3 heads
for 2048 query rows against all 4096 keys, fused (scores never hit HBM).

Host-side sharding prep packs each core's inputs in their on-chip layout
(bf16, transposed hidden states, chunk-major weights), so the device spends no
time on layout transforms.

Per-core dataflow (bf16 matmuls, fp32 PSUM accumulation), engineered so every
matmul runs in the same 128x128 PE tiling mode (tiling-mode switches drain the
TensorE; only the 48 output transposes differ):
  - heads live on complementary partition halves: Q^T/K^T for heads 0/2 occupy
    partitions 0:64 (upper half zeroed), head 1 occupies 64:128 (lower half
    zeroed). Score matmuls contract the full 128 partitions; the zero half
    contributes nothing, so tile_size stays 128x128.
  - paired projection matmuls produce two heads per instruction (head 0 cols
    0:64 + head 1 cols 64:128 of the stationary weights).
  - scores land transposed (S^T[k, q]) in PSUM; one ScalarE Exp per
    [128, 1024] tile writes bf16 P^T straight to SBUF (scale=1/8 folded in).
  - additive attention mask handled exactly by scaling V rows (and the
    appended ones-column) with exp(mask[k]) computed on device.
  - V is augmented with a ones column per head, so the context matmul
    accumulates both sum(p*v) and sum(p) (the softmax denominator) in one
    PSUM group.
  - ctx^T [65, 512] tiles are PE-transposed back to [q, d] layout, divided by
    the denominator on VectorE, and DMA'd out.
  - projection/V work is hand-interleaved into the attention sweep so the PE
    fills activation bubbles instead of serializing up front.
"""

import sys

sys.path.insert(0, "/opt/trn_rl_repo")

import ml_dtypes
import numpy as np

import concourse.bacc as bacc
import concourse.mybir as mybir
import concourse.tile as tile
from concourse import bass_utils

B, S, HID = 1, 4096, 768
NH, HD = 12, 64
N_CORES = 8
HG = 4  # head-groups (tensor parallel)
QS = 2  # query splits (data parallel on sequence)
HPC = NH // HG  # 3 heads per core
SQ = S // QS  # 2048 query rows per core
CC = HPC * HD  # 192 projection columns per core
WCC = 256  # weight cols per chunk in wqb/wkb: [h0|h1|h2|zeros]
VC = HPC * (HD + 1)  # 195 augmented V columns (ones col per head)
NHC = HID // 128  # 6 contraction chunks
NT = S // 128  # 32 key tiles

f32 = mybir.dt.float32
bf16 = mybir.dt.bfloat16
bf16np = ml_dtypes.bfloat16

# per-head partition placement: heads 0/2 on partitions 0:64, head 1 on 64:128
H_LO = {0: 0, 1: 64, 2: 0}

_CACHE = {}


def _build():
    EXP = mybir.ActivationFunctionType.Exp
    nc = bacc.Bacc("TRN2", target_bir_lowering=False)

    hsT_d = nc.dram_tensor("hsT", [HID, S], bf16, kind="ExternalInput")
    hsqT_d = nc.dram_tensor("hsqT", [HID, SQ], bf16, kind="ExternalInput")
    wqb_d = nc.dram_tensor("wqb", [128, NHC * WCC], bf16, kind="ExternalInput")
    wkb_d = nc.dram_tensor("wkb", [128, NHC * WCC], bf16, kind="ExternalInput")
    wvb_d = nc.dram_tensor("wvb", [128, NHC * VC], bf16, kind="ExternalInput")
    bqt_d = nc.dram_tensor("bqt", [128, HPC], f32, kind="ExternalInput")
    bkt_d = nc.dram_tensor("bkt", [128, HPC], f32, kind="ExternalInput")
    bvb_d = nc.dram_tensor("bvb", [1, VC], bf16, kind="ExternalInput")
    maskt_d = nc.dram_tensor("maskt", [128, NT], f32, kind="ExternalInput")
    ident_d = nc.dram_tensor("ident", [128, 128], f32, kind="ExternalInput")
    out_d = nc.dram_tensor("out", [SQ, CC], f32, kind="ExternalOutput")

    with tile.TileContext(nc) as tc:
        with (
            tc.tile_pool(name="persist", bufs=1) as P,
            tc.tile_pool(name="work", bufs=6) as WK,
            tc.tile_pool(name="outp", bufs=2) as OP,
            tc.tile_pool(name="ppsum", bufs=2, space="PSUM") as PP,
            tc.tile_pool(name="bpsum", bufs=2, space="PSUM") as BP,
            tc.tile_pool(name="cpsum", bufs=2, space="PSUM") as CP,
        ):
            # ---- persistent SBUF tensors ----
            # chunk-major transposed activations: chunk c at cols [c*S, (c+1)*S)
            hsT = P.tile([128, NHC * S], bf16, tag="hsT")
            hsTq = P.tile([128, NHC * SQ], bf16, tag="hsTq")
            wqb = P.tile([128, NHC * WCC], bf16, tag="wqb")
            wkb = P.tile([128, NHC * WCC], bf16, tag="wkb")
            wvb = P.tile([128, NHC * VC], bf16, tag="wvb")
            bvb = P.tile([1, VC], bf16, tag="bvb")
            bqt = P.tile([128, HPC], f32, tag="bqt")
            bkt = P.tile([128, HPC], f32, tag="bkt")
            maskt = P.tile([128, NT], f32, tag="maskt")
            wmask = P.tile([128, NT], f32, tag="wmask")
            identf = P.tile([128, 128], f32, tag="identf")
            onesb = P.tile([1, 128], bf16, tag="onesb")
            qt = [
                P.tile([128, SQ], bf16, tag=f"qt{h}", name=f"qt{h}")
                for h in range(HPC)
            ]
            kt = [
                P.tile([128, S], bf16, tag=f"kt{h}", name=f"kt{h}")
                for h in range(HPC)
            ]
            vv = P.tile([128, NT * VC], bf16, tag="vv")

            # zero the unused partition half of each head's q/k tensors so
            # full-128-partition score contractions are exact
            for h in range(HPC):
                lo = H_LO[h]
                z0, z1 = (64, 128) if lo == 0 else (0, 64)
                nc.vector.memset(qt[h][z0:z1, :], 0.0)
                nc.vector.memset(kt[h][z0:z1, :], 0.0)

            # ---- emission helpers ----
            hsT_3d = hsT.rearrange("p (c s) -> p c s", s=S)
            hsT_d3 = hsT_d.rearrange("(c p) s -> p c s", p=128)
            hsTq_3d = hsTq.rearrange("p (c s) -> p c s", s=SQ)
            hsqT_d3 = hsqT_d.rearrange("(c p) s -> p c s", p=128)

            def load_hsT_cols(s0, s1):
                nc.sync.dma_start(hsT_3d[:, :, s0:s1], hsT_d3[:, :, s0:s1])

            def load_hsqT_cols(s0, s1):
                nc.sync.dma_start(hsTq_3d[:, :, s0:s1], hsqT_d3[:, :, s0:s1])

            # projection units: the h0/h1 pair shares one matmul chain
            # (stationary cols 0:128 of the chunk), h2 uses cols 128:256
            # (top 64 of those are zero padding)
            qt_done = set()

            def qt_unit(h, j):
                key = (0 if h in (0, 1) else 2, j)
                if key in qt_done:
                    return
                qt_done.add(key)
                coff = 0 if key[0] == 0 else 128
                pq = PP.tile([128, 512], f32, tag="proj", name="pq")
                for c in range(NHC):
                    nc.tensor.matmul(
                        pq[:],
                        wqb[:, c * WCC + coff : c * WCC + coff + 128],
                        hsTq[:, c * SQ + j * 512 : c * SQ + (j + 1) * 512],
                        start=(c == 0),
                        stop=(c == NHC - 1),
                    )
                if key[0] == 0:
                    nc.vector.tensor_scalar_add(
                        qt[0][0:64, j * 512 : (j + 1) * 512],
                        pq[0:64, :],
                        bqt[0:64, 0:1],
                    )
                    nc.vector.tensor_scalar_add(
                        qt[1][64:128, j * 512 : (j + 1) * 512],
                        pq[64:128, :],
                        bqt[64:128, 1:2],
                    )
                else:
                    nc.vector.tensor_scalar_add(
                        qt[2][0:64, j * 512 : (j + 1) * 512],
                        pq[0:64, :],
                        bqt[0:64, 2:3],
                    )

            kt_done = set()

            def kt_unit(h, j):
                # produces key block [512j, 512(j+1)) for the h0/h1 pair or h2
                key = (0 if h in (0, 1) else 2, j)
                if key in kt_done:
                    return
                kt_done.add(key)
                coff = 0 if key[0] == 0 else 128
                pk = PP.tile([128, 512], f32, tag="proj", name="pk")
                for c in range(NHC):
                    nc.tensor.matmul(
                        pk[:],
                        wkb[:, c * WCC + coff : c * WCC + coff + 128],
                        hsT[:, c * S + j * 512 : c * S + (j + 1) * 512],
                        start=(c == 0),
                        stop=(c == NHC - 1),
                    )
                if key[0] == 0:
                    nc.vector.tensor_scalar_add(
                        kt[0][0:64, j * 512 : (j + 1) * 512],
                        pk[0:64, :],
                        bkt[0:64, 0:1],
                    )
                    nc.vector.tensor_scalar_add(
                        kt[1][64:128, j * 512 : (j + 1) * 512],
                        pk[64:128, :],
                        bkt[64:128, 1:2],
                    )
                else:
                    nc.vector.tensor_scalar_add(
                        kt[2][0:64, j * 512 : (j + 1) * 512],
                        pk[0:64, :],
                        bkt[0:64, 2:3],
                    )

            def v_unit(t):
                pv = PP.tile([128, VC], f32, tag="proj", name="pv")
                for c in range(NHC):
                    nc.tensor.matmul(
                        pv[:],
                        hsT[:, c * S + t * 128 : c * S + (t + 1) * 128],
                        wvb[:, c * VC : (c + 1) * VC],
                        start=(c == 0),
                        stop=False,
                    )
                nc.tensor.matmul(pv[:], onesb[:], bvb[:], start=False, stop=True)
                nc.vector.tensor_scalar_mul(
                    vv[:, t * VC : (t + 1) * VC], pv[:], wmask[:, t : t + 1]
                )

            # ---- ramp: pipelined input loads + first-needed projections ----
            # mask load + exp first: the ScalarE is in-order, so this tiny
            # ACTIVATE must clear the queue before the first score exp; its
            # DMA must not sit behind the big activation transfers
            nc.sync.dma_start(maskt[:], maskt_d[:])
            nc.scalar.activation(wmask[:], maskt[:], EXP)
            nc.vector.memset(onesb[:], 1.0)
            load_hsqT_cols(0, 512)  # enough for qt(*, 0)
            nc.sync.dma_start(wqb[:], wqb_d[:])
            nc.sync.dma_start(bqt[:], bqt_d[:])
            load_hsT_cols(0, 1024)
            nc.sync.dma_start(wkb[:], wkb_d[:])
            nc.sync.dma_start(bkt[:], bkt_d[:])
            qt_unit(0, 0)
            kt_unit(0, 0)
            load_hsT_cols(1024, 2048)
            nc.sync.dma_start(wvb[:], wvb_d[:])
            nc.sync.dma_start(bvb[:], bvb_d[:])
            nc.sync.dma_start(identf[:], ident_d[:])
            load_hsT_cols(2048, 4096)
            load_hsqT_cols(512, SQ)

            # stepwise projection queues: one matmul per gg so unit
            # bursts never overrun the per-gg ScalarE slack
            qt_q = []
            kt_q = []

            def proj_step():
                q = qt_q if qt_q else kt_q
                if not q:
                    return
                st = q[0]
                c = st["step"]
                kind, key = st["kind"], st["key"]
                coff = 0 if key[0] == 0 else 128
                if c == 0:
                    st["ps"] = PP.tile([128, 512], f32, tag="proj", name="ps")
                ps = st["ps"]
                if kind == "qt":
                    nc.tensor.matmul(
                        ps[:],
                        wqb[:, c * WCC + coff : c * WCC + coff + 128],
                        hsTq[:, c * SQ + key[1] * 512 : c * SQ + (key[1] + 1) * 512],
                        start=(c == 0),
                        stop=(c == NHC - 1),
                    )
                else:
                    nc.tensor.matmul(
                        ps[:],
                        wkb[:, c * WCC + coff : c * WCC + coff + 128],
                        hsT[:, c * S + key[1] * 512 : c * S + (key[1] + 1) * 512],
                        start=(c == 0),
                        stop=(c == NHC - 1),
                    )
                if c == NHC - 1:
                    dst = qt if kind == "qt" else kt
                    bias = bqt if kind == "qt" else bkt
                    j = key[1]
                    if key[0] == 0:
                        nc.vector.tensor_scalar_add(
                            dst[0][0:64, j * 512 : (j + 1) * 512],
                            ps[0:64, :],
                            bias[0:64, 0:1],
                        )
                        nc.vector.tensor_scalar_add(
                            dst[1][64:128, j * 512 : (j + 1) * 512],
                            ps[64:128, :],
                            bias[64:128, 1:2],
                        )
                    else:
                        nc.vector.tensor_scalar_add(
                            dst[2][0:64, j * 512 : (j + 1) * 512],
                            ps[0:64, :],
                            bias[0:64, 2:3],
                        )
                    q.pop(0)
                    return
                st["step"] += 1

            def enqueue_qt(h, j):
                key = (0 if h in (0, 1) else 2, j)
                if key in qt_done:
                    return
                qt_done.add(key)
                qt_q.append({"kind": "qt", "key": key, "step": 0})

            def enqueue_kt(h, j):
                key = (0 if h in (0, 1) else 2, j)
                if key in kt_done:
                    return
                kt_done.add(key)
                kt_q.append({"kind": "kt", "key": key, "step": 0})

            # deferred out-stage, pipelined into the next block's g-loop
            out_stage_q = []

            def emit_out_stage():
                if not out_stage_q:
                    return
                jq, h, cx, st = out_stage_q[0]
                if st["step"] == 0:
                    cs = OP.tile([65, 512], f32, tag="cs", name="cs")
                    nc.vector.tensor_copy(cs[:], cx[:])
                    st["cs"] = cs
                    st["ot"] = OP.tile([128, 4 * 64], f32, tag="ot", name="ot")
                elif st["step"] == 1:
                    # all four transposes back-to-back: only two PE
                    # tiling-mode switches per block instead of eight
                    cs = st["cs"]
                    tp2 = PP.tile([128, 4 * 65], f32, tag="proj", name="tp2")
                    st["tp2"] = tp2
                    for t4 in range(4):
                        nc.tensor.transpose(
                            tp2[:, t4 * 65 : (t4 + 1) * 65],
                            cs[:, t4 * 128 : (t4 + 1) * 128],
                            identf[0:65, 0:65],
                        )
                elif st["step"] <= 5:
                    t4 = st["step"] - 2
                    tp2, ot = st["tp2"], st["ot"]
                    rc = OP.tile([128, 1], f32, tag="rc", name="rc")
                    nc.vector.reciprocal(rc[:], tp2[:, t4 * 65 + 64 : t4 * 65 + 65])
                    nc.vector.tensor_scalar_mul(
                        ot[:, t4 * 64 : (t4 + 1) * 64],
                        tp2[:, t4 * 65 : t4 * 65 + 64],
                        rc[:],
                    )
                    if t4 == 3:
                        dst = out_d[
                            jq * 512 : (jq + 1) * 512, h * 64 : (h + 1) * 64
                        ].rearrange("(t p) d -> p t d", p=128)
                        nc.sync.dma_start(
                            dst, ot.rearrange("p (t d) -> p t d", d=64)
                        )
                        out_stage_q.pop(0)
                        return
                st["step"] += 1

            def flush_out_stages():
                while out_stage_q:
                    emit_out_stage()

            # ---- attention sweep (head-outer for projection spreading) ----
            blocks = [(jq, h) for h in range(HPC) for jq in range(SQ // 512)]
            pending_final = None

            for bi, (jq, h) in enumerate(blocks):
                qt_unit(h, jq)
                cx = CP.tile([65, 512], f32, tag="ctx", name="cx")
                pts = []

                def emit_ctx(pgg, cx=cx, pts=pts, h=h):
                    pt = pts[pgg]
                    for gi, g in enumerate((2 * pgg, 2 * pgg + 1)):
                        nc.tensor.matmul(
                            cx[:],
                            vv[:, g * VC + h * 65 : g * VC + h * 65 + 65],
                            pt[:, gi * 512 : (gi + 1) * 512],
                            start=(g == 0),
                            stop=(g == 31),
                        )

                for gg in range(16):
                    # scores for key chunks 2gg, 2gg+1 -> one [128, 1024] exp
                    sc = BP.tile([128, 1024], f32, tag="big", name="sc")
                    for gi, g in enumerate((2 * gg, 2 * gg + 1)):
                        nc.tensor.matmul(
                            sc[:, gi * 512 : (gi + 1) * 512],
                            kt[h][:, g * 128 : (g + 1) * 128],
                            qt[h][:, jq * 512 : (jq + 1) * 512],
                            start=True,
                            stop=True,
                        )
                    pt = WK.tile([128, 1024], bf16, tag="pts", name="pt")
                    nc.scalar.activation(pt[:], sc[:], EXP, scale=0.125)
                    pts.append(pt)
                    if gg == 0 and pending_final is not None:
                        pending_final()
                        pending_final = None
                    emit_out_stage()
                    # interleave remaining projection work into the
                    # activation-bound steady state (after the exp emission so
                    # scores are never delayed behind projection work)
                    if bi == 0:
                        v_unit(2 * gg)
                        v_unit(2 * gg + 1)
                        if gg % 2 == 0 and gg // 2 + 1 <= 7:
                            kt_unit(0, gg // 2 + 1)
                    else:
                        if gg == 0 and bi == 1:
                            for j2 in range(8):
                                enqueue_kt(2, j2)
                        if gg == 6 and bi + 1 < len(blocks):
                            njq, nh = blocks[bi + 1]
                            enqueue_qt(nh, njq)
                        if gg == 10 and bi + 2 < len(blocks):
                            njq, nh = blocks[bi + 2]
                            enqueue_qt(nh, njq)
                        proj_step()

                    # ctx runs one step behind exp so the PE overlaps the
                    # activation latency with the previous chunk-pair's ctx
                    if gg > 0:
                        emit_ctx(gg - 1)
                # final chunk-pair's ctx is deferred into the next block so
                # the transition never stalls on the last exp
                pending_final = (lambda f=emit_ctx: f(15))
                out_stage_q.append((jq, h, cx, {"step": 0}))
            if pending_final is not None:
                pending_final()
                pending_final = None
            flush_out_stages()

    nc.compile()
    return nc


def _get_nc():
    if "nc" not in _CACHE:
        _CACHE["nc"] = _build()
    return _CACHE["nc"]


def _in_maps(hs, mask, Wq, bq, Wk, bk, Wv, bv):
    ident = np.eye(128, dtype=np.float32)
    maskt = np.ascontiguousarray(mask.reshape(NT, 128).T)  # [128, 32]
    hsT = np.ascontiguousarray(hs.astype(bf16np).T)  # [768, 4096] bf16
    hsqT = [
        np.ascontiguousarray(hs[sh * SQ : (sh + 1) * SQ, :].astype(bf16np).T)
        for sh in range(QS)
    ]

    def qk_chunks(W, hg):  # [768, :] f32 -> [128, 6*256] bf16: [h0|h1|h2|0]
        out = np.zeros((128, NHC * WCC), bf16np)
        for c in range(NHC):
            out[:, c * WCC : c * WCC + CC] = W[
                c * 128 : (c + 1) * 128, hg * CC : (hg + 1) * CC
            ].astype(bf16np)
        return out

    def v_chunks(W):  # augmented V weights -> [128, 6*195] bf16
        out = np.empty((128, NHC * VC), bf16np)
        for c in range(NHC):
            out[:, c * VC : (c + 1) * VC] = W[c * 128 : (c + 1) * 128, :].astype(
                bf16np
            )
        return out

    maps = []
    for core in range(N_CORES):
        hg, sh = core // QS, core % QS
        wv_aug = np.zeros((HID, VC), np.float32)
        bv_aug = np.zeros((1, VC), np.float32)
        for h in range(HPC):
            wv_aug[:, h * 65 : h * 65 + 64] = Wv[
                :, hg * CC + h * 64 : hg * CC + (h + 1) * 64
            ]
            bv_aug[0, h * 65 : h * 65 + 64] = bv[
                hg * CC + h * 64 : hg * CC + (h + 1) * 64
            ]
            bv_aug[0, h * 65 + 64] = 1.0
        # per-head bias columns, placed on each head's partition half
        bqt = np.zeros((128, HPC), np.float32)
        bkt = np.zeros((128, HPC), np.float32)
        for h in range(HPC):
            lo = H_LO[h]
            bqt[lo : lo + 64, h] = bq[hg * CC + h * 64 : hg * CC + (h + 1) * 64]
            bkt[lo : lo + 64, h] = bk[hg * CC + h * 64 : hg * CC + (h + 1) * 64]
        maps.append(
            {
                "hsT": hsT,
                "hsqT": hsqT[sh],
                "wqb": qk_chunks(Wq, hg),
                "wkb": qk_chunks(Wk, hg),
                "wvb": v_chunks(wv_aug),
                "bqt": bqt,
                "bkt": bkt,
                "bvb": bv_aug.astype(bf16np),
                "maskt": maskt,
                "ident": ident,
            }
        )
    return maps


def kernel(hidden_states, attention_mask, Wq, bq, Wk, bk, Wv, bv, **run_kwargs):
    hs = np.ascontiguousarray(np.asarray(hidden_states, np.float32).reshape(S, HID))
    mask = np.ascontiguousarray(np.asarray(attention_mask, np.float32).reshape(S))
    Wq = np.asarray(Wq, np.float32)
    Wk = np.asarray(Wk, np.float32)
    Wv = np.asarray(Wv, np.float32)
    bq = np.asarray(bq, np.float32)
    bk = np.asarray(bk, np.float32)
    bv = np.asarray(bv, np.float32)

    nc = _get_nc()
    maps = _in_maps(hs, mask, Wq, bq, Wk, bk, Wv, bv)
    res = bass_utils.run_bass_kernel_spmd(
        nc, maps, core_ids=list(range(N_CORES)), **run_kwargs
    )
    out = np.zeros((S, NH * HD), np.float32)
    for core in range(N_CORES):
        hg, sh = core // QS, core % QS
        out[sh * SQ : (sh + 1) * SQ, hg * CC : (hg + 1) * CC] = res.results[core][
            "out"
        ]
    if "trace" in run_kwargs:
        _CACHE["last_result"] = res
    return out.reshape(B, S, NH * HD)
